# revision 52
# baseline (speedup 1.0000x reference)
"""Causal self-attention TP kernel for 8 trn2 NeuronCores.

Problem shapes (hardcoded): x [2, 2048, 2048] f32, w_attn [2048, 6144],
w_proj [2048, 2048], 16 heads, head_dim 128.

Sharding: tensor-parallel over heads — core i owns heads {2i, 2i+1} for BOTH
batches. Each core computes its local-head qkv + attention, producing
y_local^T [512 feat, 4096 tok]. Two 8-core AllToAlls (one per batch)
re-shard from feature-split to token-split: core g receives
y^T[all 2048 feat, 256 tokens of each batch] and projects those 512 tokens
against the full w_proj, emitting out[512, 2048] (batch0 rows then batch1).
The batch-0 AllToAll overlaps batch-1 attention compute; the final
projection overlaps the batch-1 AllToAll.

Dtypes: the qkv projections run fp32r (FP22, full PE rate at free-dim
>= 256) off the f32 inputs. Everything downstream of the qkv PSUM is
bf16: q/k/v tiles, exp(p), the causal mask, y, both AllToAll buffers and
w_proj — bf16 matmuls run at full PE rate at ANY free-dim (so causal
diagonal tiles compute exact column subranges), DVE elementwise gets its
2x 16-bit mode, and the v/y/a2a/w_proj DMA + collective bytes halve.
Softmax row-sums accumulate in f32 PSUM (ones-matmul on the PE), applied
to y after the PV matmul via a partition-broadcast reciprocal.

DMA ordering: transfers execute roughly serially in issue order, so the
startup sequence is explicitly wv -> x(tt0) -> wqk -> x(tt1) ... on one
queue; the mask slots in behind tile 3. v tiles for batch 1 and all 8
bf16 w_proj chunks prefetch during attention windows.
"""

import os
import numpy as np
import ml_dtypes

import concourse.bass as bass
import concourse.mybir as mybir
import concourse.tile as tile
from concourse import bacc
from concourse.bass_utils import run_bass_kernel_spmd

F32 = mybir.dt.float32
F32R = mybir.dt.float32r
BF16 = mybir.dt.bfloat16
BF16NP = ml_dtypes.bfloat16

B, T, C = 2, 2048, 2048
H, D = 16, 128
NTOK = B * T                     # 4096 flat tokens (batch-major)
SCALE = 1.0 / float(np.sqrt(D))  # 0.08838834764831845
NCORES = 8
HPC = H // NCORES                # 2 heads per core
FLOC = HPC * D                   # 256 local v features
QK = 512                         # q+k local features (2 heads x 128 x 2)

last_exec_time_ns = None
_cache = {}


def r32(ap):
    return ap.bitcast(F32R)


def _masks_np():
    # mask[m, kk, qq] = 1.0 iff kk <= qq - 128*m   (for diagonal tile offset m)
    m = np.arange(4)[:, None, None]
    kk = np.arange(128)[None, :, None]
    qq = np.arange(512)[None, None, :]
    return (kk <= qq - 128 * m).astype(BF16NP)


def build_nc(no_collective=False, reps=1):
    nc = bacc.Bacc("TRN2", target_bir_lowering=False, debug=False,
                   num_devices=1 if no_collective else NCORES)

    xt = nc.dram_tensor("xt", [C, NTOK], BF16, kind="ExternalInput")
    wqk = nc.dram_tensor("wqk", [C, QK], BF16, kind="ExternalInput")
    wv = nc.dram_tensor("wv", [C, FLOC], BF16, kind="ExternalInput")
    wp = nc.dram_tensor("wp", [C, C], BF16, kind="ExternalInput")
    out = nc.dram_tensor("out", [512, C], F32, kind="ExternalOutput")

    v_dram = nc.dram_tensor("v_dram", [NTOK, FLOC], BF16)
    # per-batch a2a buffers: 8 shards x [256 feat x 256 tok], bf16 (halves
    # the collective wire bytes). batch-0's collective hides behind batch-1
    # attention; batch-1's tail overlaps the batch-0 projection groups.
    y_loc = [nc.dram_tensor(f"y_loc{b}", [2048, 256], BF16) for b in range(B)]
    y_t = [nc.dram_tensor(f"y_t{b}", [2048, 256], BF16) for b in range(B)]
    masks = nc.inline_tensor(_masks_np(), "masks")      # [4, 128, 512] bf16
    # [128,128] ones: the row-sum matmul then emits sums replicated across
    # all 128 partitions (matmul cost is column count, so this is free) —
    # no partition_broadcast needed, keeping attention off the Pool engine,
    # which the 41us CollectiveCompute occupies.
    ones_dr = nc.inline_tensor(np.ones((128, 128), BF16NP), "ones_c")
    zeros_dr = nc.inline_tensor(np.zeros((128, 1), np.float32), "zeros_c")

    def a2a(b):
        if no_collective:
            nc.sync.dma_start(out=y_t[b][:, :], in_=y_loc[b][:, :])
        else:
            nc.gpsimd.collective_compute(
                "AllToAll",
                mybir.AluOpType.bypass,
                replica_groups=[list(range(NCORES))],
                ins=[y_loc[b][:, :]],
                outs=[y_t[b][:, :]],
            )

    with tile.TileContext(nc) as tc:
      for _rep in range(reps):
        with (
            tc.tile_pool(name="persist", bufs=1) as persist,
            tc.tile_pool(name="psA", bufs=4, space="PSUM") as psA,
            tc.tile_pool(name="psY", bufs=2, space="PSUM") as psY,
            tc.tile_pool(name="psR", bufs=2, space="PSUM") as psR,
        ):
            ones_sb = persist.tile([128, 128], BF16)
            zeros_sb = persist.tile([128, 1], F32)
            nc.gpsimd.dma_start(out=zeros_sb, in_=zeros_dr.ap())
            scr = persist.tile([128, 1], F32)
            # warm the ACT exp table set (~2.7us) before attention needs it
            nc.scalar.activation(scr, zeros_sb,
                                 mybir.ActivationFunctionType.Exp, bias=zeros_sb)
            nc.gpsimd.dma_start(out=ones_sb, in_=ones_dr.ap())

            with (
                tc.tile_pool(name="pattn", bufs=1) as pattn,
                tc.tile_pool(name="p2v", bufs=4) as p2v,
                tc.tile_pool(name="p2p", bufs=4) as p2p,
                tc.tile_pool(name="p2y", bufs=2) as p2y,
                tc.tile_pool(name="p2r", bufs=3) as p2r,
                # projection pools open alongside phase 1: address-disjoint
                # from the p1 pools, so no SBUF-reuse barrier fires mid-kernel
                tc.tile_pool(name="p4w", bufs=8) as p4w,
                tc.tile_pool(name="p4y", bufs=1) as p4y,
                tc.tile_pool(name="p4s", bufs=2) as p4s,
            ):
                # q^T,k^T for 2 heads, all tokens: chunk f = {q_h0,q_h1,k_h0,k_h1}
                qk_res = pattn.tile([128, 4, NTOK], BF16)
                mask_sb = pattn.tile([128, 4, 512], BF16)
                v_pre = {}

                def load_v(b, h):
                    v_sb = p2v.tile([128, 16, 128], BF16, tag="vsb",
                                    name=f"v_sb{b}{h}")
                    nc.gpsimd.dma_start(
                        out=v_sb,
                        in_=v_dram[b * T:(b + 1) * T,
                                   h * 128:(h + 1) * 128].rearrange(
                            "(n p) f -> p n f", p=128),
                    )
                    return v_sb

                # ================= phase 1: qkv =================
                with (
                    tc.tile_pool(name="p1w", bufs=1) as p1w,
                    tc.tile_pool(name="p1x", bufs=4) as p1x,
                    tc.tile_pool(name="p1s", bufs=1) as p1s,
                ):
                    wqk_sb = p1w.tile([128, 16, QK], BF16)
                    wv_sb = p1w.tile([128, 16, FLOC], BF16)
                    # weights stream on the Pool (gpsimd) queue, concurrent
                    # with the x stream on the SP queue. Quarter-granularity
                    # for wv and tile 0's x: the first v-matmul only needs
                    # wv q0 + x q0 (~2.7us) instead of full halves (~5us).
                    for q in range(4):
                        nc.gpsimd.dma_start(
                            out=wv_sb[:, q * 4:(q + 1) * 4, :],
                            in_=wv[q * 512:(q + 1) * 512, :]
                            .rearrange("(n p) f -> p n f", p=128))

                    for tt in range(8):          # 512-token tiles, 4096 tokens
                        # halves: 4 quarters would pin all p1x buffers to
                        # tile end (every psum group reads every part),
                        # blocking the next tile's prefetch
                        nparts = 2
                        cpp = 16 // nparts
                        xh = []
                        for part in range(nparts):
                            xbuf = p1x.tile([128, cpp, 512], BF16, tag="xh")
                            c0 = part * cpp
                            nc.sync.dma_start(
                                out=xbuf,
                                in_=xt[c0 * 128:(c0 + cpp) * 128,
                                       tt * 512:(tt + 1) * 512].rearrange(
                                    "(n p) t -> p n t", p=128),
                            )
                            xh.append(xbuf)
                        if tt == 0:
                            for half in range(2):
                                nc.gpsimd.dma_start(
                                    out=wqk_sb[:, half * 8:(half + 1) * 8, :],
                                    in_=wqk[half * 1024:(half + 1) * 1024, :]
                                    .rearrange("(n p) f -> p n f", p=128))
                        elif tt == 3:
                            # mask is first needed at attention start; slot it
                            # behind the weights on the Pool queue
                            nc.gpsimd.dma_start(
                                out=mask_sb,
                                in_=masks.ap().rearrange("m p q -> p m q"))
                        # v token blocks first (token-major out)
                        if tt >= 4:
                            vst = p1s.tile([128, 4, FLOC], BF16, tag="vst")
                        elif tt == 0:
                            for h in range(HPC):
                                v_pre[(0, h)] = p2v.tile(
                                    [128, 16, 128], BF16, tag="vsb",
                                    name=f"v_pre0{h}")
                        for tb in range(4):
                            psv = psA.tile([128, FLOC], F32, tag="ps")
                            for c in range(16):
                                nc.tensor.matmul(
                                    psv,
                                    lhsT=xh[c // cpp][:, c % cpp,
                                                      tb * 128:(tb + 1) * 128],
                                    rhs=wv_sb[:, c, :],
                                    start=(c == 0), stop=(c == 15),
                                )
                            if tt < 4:
                                # batch-0 v goes straight to its attention tiles
                                for h in range(HPC):
                                    nc.vector.tensor_copy(
                                        v_pre[(0, h)][:, tt * 4 + tb, :],
                                        psv[:, h * 128:(h + 1) * 128])
                            else:
                                nc.vector.tensor_copy(vst[:, tb, :], psv)
                        if tt >= 4:
                            nc.sync.dma_start(
                                out=v_dram[tt * 512:(tt + 1) * 512, :].rearrange(
                                    "(tb p) f -> p tb f", p=128),
                                in_=vst,
                            )
                        # q^T / k^T feature blocks
                        for fb in range(4):
                            ps = psA.tile([128, 512], F32, tag="ps")
                            for c in range(16):
                                nc.tensor.matmul(
                                    ps,
                                    lhsT=wqk_sb[:, c, fb * 128:(fb + 1) * 128],
                                    rhs=xh[c // cpp][:, c % cpp, :],
                                    start=(c == 0), stop=(c == 15),
                                )
                            nc.vector.tensor_copy(
                                qk_res[:, fb, tt * 512:(tt + 1) * 512], ps)

                # ============ phases 2+3: attention + per-batch a2a ============
                if True:
                    def load_wp_chunk(ch):
                        wt = p4w.tile([128, 16, 256], BF16, tag="wp",
                                      name=f"wp_t{ch}")
                        nc.gpsimd.dma_start(
                            out=wt,
                            in_=wp[:, ch * 256:(ch + 1) * 256].rearrange(
                                "(n p) f -> p n f", p=128))
                        return wt

                    def load_yt(b):
                        # yt0 stays whole on the Pool queue (SP/ACT would
                        # head-of-line block critical traffic behind its a2a
                        # wait). yt1 is on the critical tail: split halves
                        # across Pool and SP (both idle by then) so the two
                        # 0.5MB transfers run in parallel.
                        yb = p4y.tile([128, 16, 256], BF16, tag=f"yt{b}",
                                      name=f"yt{b}")
                        src = y_t[b].ap().rearrange("(n p) t -> p n t", p=128)
                        if b == 0:
                            nc.gpsimd.dma_start(out=yb, in_=src)
                        else:
                            nc.gpsimd.dma_start(out=yb[:, 0:8, :],
                                                in_=src[:, 0:8, :])
                            nc.sync.dma_start(out=yb[:, 8:16, :],
                                              in_=src[:, 8:16, :])
                        return yb

                    # y_loc[b] rows: (2j+e)*256 + h*128 + d  ->  [j,h,d,e,q]
                    yloc_ap = [
                        y_loc[b].ap().rearrange(
                            "(j e h d) q -> j h d e q", j=4, e=2, h=2, d=128)
                        for b in range(B)
                    ]

                    def proj_group(yts, wt, b, ch):
                        st = p4s.tile([128, 2, 256], F32, tag="ost")
                        for tb in range(2):
                            ps = psA.tile([128, 256], F32, tag="ps")
                            for c in range(16):
                                nc.tensor.matmul(
                                    ps,
                                    lhsT=yts[b][:, c, tb * 128:(tb + 1) * 128],
                                    rhs=wt[:, c, :],
                                    start=(c == 0), stop=(c == 15),
                                )
                            nc.vector.tensor_copy(st[:, tb, :], ps)
                        # batch-0 stores go via the ACT queue (idle after
                        # attention): on SP they would queue behind yt1's
                        # half-load waiting on the batch-1 a2a
                        eng = nc.scalar if b == 0 else nc.sync
                        eng.dma_start(
                            out=out[b * 256:(b + 1) * 256,
                                    ch * 256:(ch + 1) * 256].rearrange(
                                "(tb p) f -> p tb f", p=128),
                            in_=st,
                        )

                    # prefetches keyed on (b, h, j): batch-1 v tiles early in
                    # batch 0's window; all 8 bf16 w_proj chunks spread over
                    # both attention windows (64KB total in p4w).
                    wp_tiles = []

                    def wp_pre(ch):
                        def f():
                            while len(wp_tiles) <= ch:
                                wp_tiles.append(load_wp_chunk(len(wp_tiles)))
                        return f

                    # everything prefetches inside batch 0's window: its DMA
                    # slack absorbs all of it, so batch 1's window carries
                    # only its own y stores — the last store (which gates the
                    # critical batch-1 a2a) is never queued behind bulk loads
                    prefetch = {
                        (0, 0, 1): lambda: v_pre.setdefault((1, 0), load_v(1, 0)),
                        (0, 0, 2): lambda: v_pre.setdefault((1, 1), load_v(1, 1)),
                        (0, 0, 3): wp_pre(0),
                        (0, 1, 0): wp_pre(1),
                        (0, 1, 1): wp_pre(3),
                        (0, 1, 2): wp_pre(5),
                        (0, 1, 3): wp_pre(7),
                    }

                    yts = []
                    for b in range(B):
                        for h in range(HPC):
                            v_sb = v_pre.pop((b, h), None) or load_v(b, h)
                            qf, kf = h, 2 + h
                            tok0 = b * T
                            # last head runs j in reverse: the batch's final
                            # iteration is then the 4-tile j=0, shortening the
                            # exp->mask->y/r->recip->store chain that gates
                            # the batch's a2a launch
                            jorder = range(4) if h == 0 else (3, 2, 1, 0)
                            for j in jorder:
                                if (b, h, j) in prefetch:
                                    prefetch[(b, h, j)]()
                                nk = 4 * j + 4
                                y_ps = psY.tile([128, 512], F32, tag="yps")
                                r_ps = psR.tile([128, 512], F32, tag="rps")
                                qs = qk_res[:, qf,
                                            tok0 + j * 512: tok0 + (j + 1) * 512]
                                for c in range(nk):
                                    m = c - 4 * j
                                    # diagonal tiles: only the valid columns
                                    # (bf16 matmul is full-rate at any width)
                                    q0 = 128 * m if m > 0 else 0
                                    s_ps = psA.tile([128, 512], F32, tag="ps")
                                    nc.tensor.matmul(
                                        s_ps[:, q0:],
                                        lhsT=qk_res[:, kf,
                                                    tok0 + c * 128:
                                                    tok0 + (c + 1) * 128],
                                        rhs=qs[:, q0:],
                                        start=True, stop=True,
                                    )
                                    p_sb = p2p.tile([128, 512], BF16, tag="p")
                                    nc.scalar.activation(
                                        p_sb[:, q0:], s_ps[:, q0:],
                                        mybir.ActivationFunctionType.Exp,
                                        scale=SCALE, bias=zeros_sb,
                                    )
                                    if m >= 0:
                                        nc.vector.tensor_mul(
                                            p_sb[:, q0:], p_sb[:, q0:],
                                            mask_sb[:, m, q0:])
                                    nc.tensor.matmul(
                                        y_ps[:, q0:],
                                        lhsT=v_sb[:, c, :],
                                        rhs=p_sb[:, q0:],
                                        start=(c == 0), stop=(c == nk - 1),
                                    )
                                    nc.tensor.matmul(
                                        r_ps[:, q0:],
                                        lhsT=ones_sb,
                                        rhs=p_sb[:, q0:],
                                        start=(c == 0), stop=(c == nk - 1),
                                    )
                                rr = p2r.tile([128, 512], F32, tag="rr")
                                nc.vector.reciprocal(rr, r_ps)
                                y_sb = p2y.tile([128, 512], BF16, tag="ysb")
                                nc.vector.tensor_mul(y_sb, y_ps, rr)
                                nc.sync.dma_start(
                                    out=yloc_ap[b][j, h],
                                    in_=y_sb.rearrange("d (e q) -> d e q", e=2),
                                )
                        # batch-b all-to-all; b=0's overlaps b=1 attention
                        a2a(b)
                        yts.append(load_yt(b))

                    # projection: all batch-0 chunks first (they only need the
                    # batch-0 a2a); batch-1 follows while its a2a drains
                    for b in range(B):
                        for ch in range(8):
                            while len(wp_tiles) <= ch:
                                wp_tiles.append(load_wp_chunk(len(wp_tiles)))
                            proj_group(yts, wp_tiles[ch], b, ch)

    nc.compile()
    return nc


def make_in_maps(x, w_attn, w_proj):
    """Per-core input dict list shared by kernel() and the test harnesses."""
    x = np.asarray(x, dtype=np.float32)
    w_attn = np.asarray(w_attn, dtype=np.float32)
    wp = np.asarray(w_proj, dtype=np.float32).astype(BF16NP)
    xt = np.ascontiguousarray(x.reshape(NTOK, C).T).astype(BF16NP)
    in_maps = []
    for i in range(NCORES):
        qcols = w_attn[:, FLOC * i: FLOC * (i + 1)]
        kcols = w_attn[:, C + FLOC * i: C + FLOC * (i + 1)]
        vcols = w_attn[:, 2 * C + FLOC * i: 2 * C + FLOC * (i + 1)]
        in_maps.append({
            "xt": xt,
            "wqk": np.ascontiguousarray(
                np.concatenate([qcols, kcols], axis=1)).astype(BF16NP),
            "wv": np.ascontiguousarray(vcols).astype(BF16NP),
            "wp": wp,
        })
    return in_maps


def kernel(x, w_attn, w_proj):
    global last_exec_time_ns
    if "nc" not in _cache:
        _cache["nc"] = build_nc()
    nc = _cache["nc"]

    in_maps = make_in_maps(x, w_attn, w_proj)
    res = run_bass_kernel_spmd(nc, in_maps, list(range(NCORES)))
    last_exec_time_ns = res.exec_time_ns

    return assemble([res.results[g]["out"] for g in range(NCORES)])


def assemble(outs):
    # core g's out rows: [0:256] = batch0 tokens [256g:256(g+1)],
    #                    [256:512] = batch1 tokens [256g:256(g+1)]
    full = np.empty((B, T, C), np.float32)
    for g in range(NCORES):
        for b in range(B):
            full[b, 256 * g: 256 * (g + 1), :] = outs[g][b * 256:(b + 1) * 256]
    return full
